# revision 2
# baseline (speedup 1.0000x reference)
"""SS2D CrossBlock kernel for 8 NeuronCores (Trainium2) — restructured.

Sharding: core c handles (b = c//2, d-half = c%2), as the baseline.

Key changes vs baseline:
- All four directions run on contiguous (de-strided) layouts: xc is
  transposed once per half for k=1,3; their y accumulates in a w-major
  PSUM tile that is transpose-added at the end.
- Scan stage fused per half-direction chunk [128, 6*1024] with
  segment-reset (zeroed a columns at segment starts).
- u is replicated to scan rows by DMA doubling (channel-permuted
  storage p = 48*(t//6) + 6*j + (t%6) for slot (t,j) = channel 8t+j),
  so b_t is a fast all-SBUF TT instead of an STT from PSUM.
- B/C replicated to 128 rows by DMA doubling; their per-segment reuse
  is a stride-0 broadcast view.
- delta broadcast via PE (bcm matmul) into PSUM; exp on Act with
  per-(k,t) scale columns -> a_t fp16 in SBUF.
- Softplus/Silu single-op activations.
"""
import numpy as np
import ml_dtypes
from contextlib import ExitStack
BF_NP = np.float16

import concourse.bass as bass
import concourse.bacc as bacc_mod
import concourse.tile as tile
from concourse import mybir
from concourse.bass_utils import run_bass_kernel_spmd

F32 = mybir.dt.float32
BF = mybir.dt.float16
AF = mybir.ActivationFunctionType
OP = mybir.AluOpType

B, HH, WW, DM = 4, 32, 32, 96
DI, NS, RD, K, L = 192, 16, 6, 4, 1024
DH = 96            # channels per core (d-half)
NT = 12            # scan tiles per direction
NSEG = 6           # tiles fused per chunk
NCH = 2            # chunks per direction
SW = NSEG * L      # fused chunk width (6144)
EPS = 1e-5

_NC = None


def store_perm():
    """P[p] = local channel stored at partition p."""
    P = np.zeros(DH, np.int64)
    for p in range(DH):
        blk, w = p // 48, p % 48
        j, t6 = w // 6, w % 6
        t = 6 * blk + t6
        P[p] = 8 * t + j
    return P


def _patch_act_tables():
    """Make Exp and Ln resolve to the combined natural_log_exp set so the
    per-direction softplus chain doesn't ping-pong activation tables.
    Set ids (dict order) are preserved; only membership of the two
    single-function sets is reduced."""
    import concourse.bacc as _bacc
    from concourse.hw_specs import get_activation_tables as _gat
    def _gat2(arch):
        d = _gat(arch)
        d2 = {}
        for name, s in d.items():
            s2 = set(s)
            if name == "exp_and_others":
                s2.discard(AF.Exp)
            if name == "natural_log":
                s2.discard(AF.Ln)
            d2[name] = s2
        return d2
    _bacc.get_activation_tables = _gat2


def build():
    _patch_act_tables()
    nc = bacc_mod.Bacc(trn_type="TRN2", target_bir_lowering=False,
                       debug=False, num_devices=8)

    def din(name, shape):
        return nc.dram_tensor(name, shape, F32, kind="ExternalInput")

    def dbf(name, shape):
        return nc.dram_tensor(name, shape, BF, kind="ExternalInput")

    xT = dbf("xT", [DM, L])                  # x[b] transposed (dm, l)
    w_xi = dbf("w_xi", [DM, DI])             # in_proj lhsT for xi (2x96 blocks)
    w_z = dbf("w_z", [DM, DH])               # in_proj lhsT for this core's z
    convd = dbf("convd", [DH, 2 * 9 * DH])   # diag conv lhsT per (half, tap)
    convb = din("convb", [DH, 2])
    xpw = dbf("xpw", [DH, K * 2 * 64])       # x_dbl lhsT packed
    dtw = dbf("dtw", [RD, K * DH])           # dt lhsT per k: [6, 96]
    dtb = din("dtb", [DH, K])                # dt bias per k (col k)
    app = din("app", [128, K * NT])          # exp scale A col per (k,t)
    bcm = dbf("bcm", [DH, NT * 128])         # delta broadcast lhsT per t
    red = dbf("red", [128, NT * DH])         # hC reduce lhsT per t
    dsum = din("dsum", [DH, 1])              # sum_k Ds
    gam = din("gam", [DH, 1])
    bet = din("bet", [DH, 1])
    wout = dbf("wout", [DH, DM])             # out_proj lhsT slice
    ones96 = dbf("ones96", [DH, 2])          # ones cols for stats matmuls
    sel2 = din("sel2", [2, 2 * DH])          # mu/inv row-select lhsT

    out_part = nc.dram_tensor("out_part", [DM, L], F32, kind="ExternalOutput")

    stats_in = nc.dram_tensor("stats_in", [2, L], F32)
    stats_out = nc.dram_tensor("stats_out", [2, L], F32)
    minv_dram = nc.dram_tensor("minv_dram", [2, L], F32)
    groups = [[0, 1], [2, 3], [4, 5], [6, 7]]

    with tile.TileContext(nc) as tc, ExitStack() as ctx:
        wpool = ctx.enter_context(tc.tile_pool(name="w", bufs=1))
        spool = ctx.enter_context(tc.tile_pool(name="s", bufs=1))
        kpool = ctx.enter_context(tc.tile_pool(name="kk", bufs=2))
        bigp = ctx.enter_context(tc.tile_pool(name="big", bufs=2))
        big1 = ctx.enter_context(tc.tile_pool(name="big1", bufs=1))
        ubcp = ctx.enter_context(tc.tile_pool(name="ubc", bufs=3))
        espp = ctx.enter_context(tc.tile_pool(name="esp", bufs=2))
        k4pool = ctx.enter_context(tc.tile_pool(name="k4", bufs=1))
        ppool = ctx.enter_context(tc.tile_pool(name="pp", bufs=1, space="PSUM"))
        ypool = ctx.enter_context(tc.tile_pool(name="yy", bufs=1, space="PSUM"))

        def load(shape, src, name, dt=F32):
            t = wpool.tile(shape, dt, tag=name, name=name + "_sb")
            nc.sync.dma_start(t[:], src[:])
            return t

        # ---- weight loads ----
        xT_sb = load([DM, L], xT, "xTs", BF)
        w_xi_sb = load([DM, DI], w_xi, "w_xi", BF)
        w_z_sb = load([DM, DH], w_z, "w_z", BF)
        convd_sb = load([DH, 2 * 9 * DH], convd, "convd", BF)
        convb_sb = load([DH, 2], convb, "convb")
        xpw_sb = load([DH, K * 2 * 64], xpw, "xpw", BF)
        dtw_sb = load([RD, K * DH], dtw, "dtw", BF)
        dtb_sb = load([DH, K], dtb, "dtb")
        app_sb = load([128, K * NT], app, "app")
        bcm_sb = load([DH, NT * 128], bcm, "bcm", BF)
        red_sb = load([128, NT * DH], red, "red", BF)
        dsum_sb = load([DH, 1], dsum, "dsum")
        gam_sb = load([DH, 1], gam, "gam")
        bet_sb = load([DH, 1], bet, "bet")
        wout_sb = load([DH, DM], wout, "wout", BF)
        ones_sb = load([DH, 2], ones96, "ones96", BF)
        sel2_sb = load([2, 2 * DH], sel2, "sel2")

        # ---- phase 1: in_proj ----
        PADL = 34 * 34 + 4
        sg = spool.tile([DH, L], BF)
        xpad_ctx = tc.tile_pool(name="xpad", bufs=1)
        xpadp = xpad_ctx.__enter__()
        xpad = [xpadp.tile([DH, PADL], BF, name=f"xpad{i}") for i in range(2)]
        for cblk in range(2):
            nc.vector.memset(xpad[cblk][:], 0.0)
        pp = 0
        for cblk in range(2):
            for h in range(2):
                psf = ppool.tile([128, 512], F32, tag=("ping0","ping1","mm0","mm1")[pp % 4],
                                 name=f"xi{cblk}{h}")
                ps = psf[0:DH, :]
                pp += 1
                nc.tensor.matmul(ps,
                                 w_xi_sb[:, cblk * DH:(cblk + 1) * DH],
                                 xT_sb[:, h * 512:(h + 1) * 512],
                                 start=True, stop=True)
                dst = xpad[cblk][:, 35:35 + 32 * 34]
                dstv = dst.rearrange("p (r c) -> p r c", r=32, c=34)[:, :, 0:32]
                half = dstv[:, h * 16:(h + 1) * 16, :]
                src = ps.rearrange("p (r c) -> p r c", r=16, c=32)
                nc.scalar.activation(half, src, AF.Copy)
        for h in range(2):
            psf = ppool.tile([128, 512], F32, tag=("ping0","ping1","mm0","mm1")[pp % 4], name=f"z{h}")
            ps = psf[0:DH, :]
            pp += 1
            nc.tensor.matmul(ps, w_z_sb[:],
                             xT_sb[:, h * 512:(h + 1) * 512],
                             start=True, stop=True)
            nc.scalar.activation(sg[:, h * 512:(h + 1) * 512], ps, AF.Silu)

        # ---- phase 2: depthwise conv + silu -> xc; transposes -> xct ----
        xc = [spool.tile([DH, L], BF, name=f"xc{i}") for i in range(2)]
        for cblk in range(2):
            for h in range(2):
                cpf = ppool.tile([128, 512], F32,
                                 tag=("ping0", "ping1")[(2 * cblk + h) % 2],
                                 name=f"cv{cblk}{h}")
                cp = cpf[0:DH, :]
                for tap in range(9):
                    dy, dx = tap // 3, tap % 3
                    st = dy * 34 + dx + 16 * h * 34
                    view = xpad[cblk][:, st:st + 16 * 34]
                    view = view.rearrange("p (r c) -> p r c", r=16, c=34)
                    view = view[:, :, 0:32]
                    w0 = (cblk * 9 + tap) * DH
                    nc.tensor.matmul(cp, convd_sb[:, w0:w0 + DH], view,
                                     start=(tap == 0), stop=(tap == 8))
                nc.scalar.activation(xc[cblk][:, h * 512:(h + 1) * 512], cp,
                                     AF.Silu, bias=convb_sb[:, cblk:cblk + 1],
                                     scale=1.0)
        xpad_ctx.__exit__(None, None, None)
        xct = [spool.tile([DH, L], BF, name=f"xct{i}") for i in range(2)]
        for cblk in range(2):
            nc.vector.tensor_copy(
                xct[cblk][:].rearrange("p (w h) -> p w h", w=32, h=32),
                xc[cblk][:].rearrange("p (h w) -> p w h", h=32, w=32))

        # ---- phase 3+4: per-direction pipeline ----
        y_rm = ypool.tile([DH, L], F32, name="y_rm")
        y_wm = ypool.tile([DH, L], F32, name="y_wm")

        def prep_compute(k, esp_acts):
            """x_dbl, dt matmuls, u; bb/cb chains on pool queue.
            esp_acts: defer the esp/Ln emission (batch across k to avoid
            activation-table ping-pong)."""
            trans = k in (1, 3)
            xs = xct if trans else xc

            # x_dbl: zk/zk2 [64, 512] PSUM
            zkf = ppool.tile([128, 512], F32, tag="mm0", name=f"zk{k}")
            zk2f = ppool.tile([128, 512], F32, tag="mm1", name=f"zk2{k}")
            zk, zk2 = zkf[0:64, :], zk2f[0:64, :]
            for h, zz in enumerate((zk, zk2)):
                for cblk in range(2):
                    w0 = (k * 2 + cblk) * 64
                    nc.tensor.matmul(
                        zz, xpw_sb[:, w0:w0 + 64],
                        xs[cblk][:, h * 512:(h + 1) * 512],
                        start=(cblk == 0), stop=(cblk == 1))
            dts = kpool.tile([RD, L], BF, tag="dts")
            bck = k4pool.tile([2 * NS, L], BF, tag=f"bck{k}")
            for h, zz in enumerate((zk, zk2)):
                nc.scalar.activation(dts[:, h * 512:(h + 1) * 512],
                                     zz[0:RD, :], AF.Copy)
                nc.scalar.activation(bck[:, h * 512:(h + 1) * 512],
                                     zz[32:64, :], AF.Copy)

            # dt matmuls -> esp (Exp) deferred via esp_acts callbacks
            dtdf = ppool.tile([128, 512], F32, tag="mm0", name=f"dtd{k}")
            dtd2f = ppool.tile([128, 512], F32, tag="mm1", name=f"dtd2{k}")
            dtd, dtd2 = dtdf[0:DH, :], dtd2f[0:DH, :]
            for h, dd in enumerate((dtd, dtd2)):
                nc.tensor.matmul(dd, dtw_sb[:, k * DH:(k + 1) * DH],
                                 dts[:, h * 512:(h + 1) * 512],
                                 start=True, stop=True)
            delta = k4pool.tile([DH, L], BF, tag=f"delta{k}")
            esp = espp.tile([DH, L], F32, tag="esp")
            for h, dd in enumerate((dtd, dtd2)):
                nc.scalar.activation(esp[:, h * 512:(h + 1) * 512], dd,
                                     AF.Exp, bias=dtb_sb[:, k:k + 1],
                                     scale=1.0)
            u = k4pool.tile([DH, L], BF, tag=f"u{k}")

            def fin():
                nc.scalar.activation(delta[:], esp[:], AF.Ln, bias=1.0,
                                     scale=1.0)
                nc.vector.tensor_tensor(u[:], delta[:], xs[0][:], OP.mult)
            esp_acts.append(fin)

            return {"delta": delta, "bck": bck, "u": u}

        def launch_ubc(st):
            # B/C replication x8 via DMA doubling on the pool queue
            bck = st["bck"]
            bb = kpool.tile([128, L], BF, tag="bb")
            cb = kpool.tile([128, L], BF, tag="cb")
            for t_, rows in ((bb, bck[0:NS, :]), (cb, bck[NS:2 * NS, :])):
                nc.gpsimd.dma_start(t_[0:16, :], rows)
                nc.gpsimd.dma_start(t_[16:32, :], t_[0:16, :])
                nc.gpsimd.dma_start(t_[32:64, :], t_[0:32, :])
                nc.gpsimd.dma_start(t_[64:128, :], t_[0:64, :])
            st["bb"], st["cb"] = bb, cb
            # u_bc chains for both chunks (doubling replication)
            u = st["u"]
            st["ubcs"] = []
            for ch in range(NCH):
                ubc = ubcp.tile([128, SW], BF, tag="ubc")
                ubc_j = ubc[:].rearrange("(j n) wl -> j n wl", j=8, n=16)
                eng = nc.sync if ch == 0 else nc.gpsimd
                eng.dma_start(ubc_j[:, 0, :], u[48 * ch:48 * ch + 48, :])
                eng.dma_start(ubc_j[:, 8, :], ubc_j[:, 0, :])
                v8 = ubc[:].rearrange("(m n8) wl -> m n8 wl", n8=8)
                eng.dma_start(v8[:, 4, :], v8[:, 0, :])
                v4 = ubc[:].rearrange("(q n4) wl -> q n4 wl", n4=4)
                eng.dma_start(v4[:, 2, :], v4[:, 0, :])
                v2 = ubc[:].rearrange("(r n2) wl -> r n2 wl", n2=2)
                eng.dma_start(v2[:, 1, :], v2[:, 0, :])
                st["ubcs"].append(ubc)

        def chunks(k, st, chs):
            trans = k in (1, 3)
            flip = k >= 2
            ypsk = y_wm if trans else y_rm
            delta, bb, cb = st["delta"], st["bb"], st["cb"]
            bbv = bb[:].unsqueeze(1).broadcast_to([128, NSEG, L])
            cbv = cb[:].unsqueeze(1).broadcast_to([128, NSEG, L])
            for ch in chs:
                ubc = st["ubcs"][ch]

                # delta broadcast (PE) -> exp (Act) -> a_t
                a_t = bigp.tile([128, SW], BF, tag="a")
                for m in range(NSEG):
                    t = 6 * ch + m
                    for h in range(2):
                        pb = ppool.tile([128, 512], F32,
                                        tag=f"ping{(2 * m + h) % 2}",
                                        name=f"bc{k}{ch}{m}{h}")
                        nc.tensor.matmul(pb[:],
                                         bcm_sb[:, t * 128:(t + 1) * 128],
                                         delta[:, h * 512:(h + 1) * 512],
                                         start=True, stop=True)
                        nc.scalar.activation(
                            a_t[:, m * L + h * 512:m * L + (h + 1) * 512],
                            pb[:], AF.Exp, bias=0.0,
                            scale=app_sb[:, k * NT + t:k * NT + t + 1])
                # segment reset
                a3 = a_t[:].rearrange("p (s l) -> p s l", s=NSEG)
                if flip:
                    nc.vector.memset(a3[:, :, L - 1:L], 0.0)
                else:
                    nc.vector.memset(a3[:, :, 0:1], 0.0)

                # b_t = u_bc * bb ; scan ; hc = h * cb
                b_t = bigp.tile([128, SW], BF, tag="b")
                nc.vector.tensor_tensor(
                    b_t[:].rearrange("p (s l) -> p s l", s=NSEG),
                    ubc[:].rearrange("p (s l) -> p s l", s=NSEG),
                    bbv, OP.mult)
                h_t = big1.tile([128, SW], BF, tag="h")
                if flip:
                    nc.vector.tensor_tensor_scan(
                        h_t[:, ::-1], a_t[:, ::-1], b_t[:, ::-1], 0.0,
                        OP.mult, OP.add)
                else:
                    nc.vector.tensor_tensor_scan(
                        h_t[:], a_t[:], b_t[:], 0.0, OP.mult, OP.add)
                hc_t = big1.tile([128, SW], BF, tag="hc")
                nc.vector.tensor_tensor(
                    hc_t[:].rearrange("p (s l) -> p s l", s=NSEG),
                    h_t[:].rearrange("p (s l) -> p s l", s=NSEG),
                    cbv, OP.mult)

                # reduce into y
                for m in range(NSEG):
                    t = 6 * ch + m
                    for h in range(2):
                        nc.tensor.matmul(
                            ypsk[:, h * 512:(h + 1) * 512],
                            red_sb[:, t * DH:(t + 1) * DH],
                            hc_t[:, m * L + h * 512:m * L + (h + 1) * 512],
                            start=(k in (0, 1) and ch == 0 and m == 0),
                            stop=(k in (2, 3) and ch == NCH - 1
                                  and m == NSEG - 1))

        # upfront prep for all directions: esp/Ln batched (k0 alone first,
        # then k1-3) to keep activation-table switches low and let the
        # bb/cb/u replication chains launch early.
        acts0, acts123 = [], []
        sts = [prep_compute(0, acts0)]
        for fin in acts0:
            fin()
        launch_ubc(sts[0])
        for k in range(1, K):
            sts.append(prep_compute(k, acts123))
        for fin in acts123:
            fin()
        for k in range(K):
            if k + 1 < K:
                launch_ubc(sts[k + 1])
            chunks(k, sts[k], (0, 1))

        # ---- phase 5: D-term + transpose-add + LN stats + AllReduce ----
        y1 = espp.tile([DH, L], F32, tag="esp")
        nc.vector.scalar_tensor_tensor(y1[:], xc[0][:], dsum_sb[:],
                                       y_rm[:], OP.mult, OP.add)
        y_full = spool.tile([DH, L], BF)
        nc.vector.tensor_tensor(
            y_full[:].rearrange("p (h w) -> p h w", h=32, w=32),
            y1[:].rearrange("p (h w) -> p h w", h=32, w=32),
            y_wm[:].rearrange("p (w h) -> p h w", w=32, h=32),
            OP.add)
        y2 = spool.tile([DH, L], BF)
        nc.scalar.activation(y2[:], y_full[:], AF.Square)
        st_y = spool.tile([1, L], F32)
        st_y2 = spool.tile([1, L], F32)
        for h in range(2):
            for row, (src_t, dst_t) in enumerate(((y_full, st_y), (y2, st_y2))):
                sspf = ppool.tile([128, 512], F32,
                                  tag=("ping0","ping1","mm0","mm1")[2 * h + row],
                                  name=f"st{h}{row}")
                ssp = sspf[0:1, :]
                nc.tensor.matmul(ssp, ones_sb[:, row:row + 1],
                                 src_t[:, h * 512:(h + 1) * 512],
                                 start=True, stop=True)
                nc.scalar.activation(dst_t[:, h * 512:(h + 1) * 512],
                                     ssp, AF.Copy)
        nc.sync.dma_start(stats_in[0:1, :], st_y[:])
        nc.sync.dma_start(stats_in[1:2, :], st_y2[:])
        nc.gpsimd.collective_compute(
            "AllReduce", OP.add, replica_groups=groups,
            ins=[stats_in[:]], outs=[stats_out[:]])
        st128 = spool.tile([128, 16], F32)
        nc.sync.dma_start(
            st128[:].rearrange("p (s f) -> p s f", s=2, f=8),
            stats_out[:].rearrange("s (p f) -> p s f", p=128, f=8))
        mu8 = spool.tile([128, 8], F32)
        nc.scalar.activation(mu8[:], st128[:, 0:8], AF.Copy, scale=1.0 / DI)
        msq = spool.tile([128, 8], F32)
        nc.scalar.activation(msq[:], st128[:, 0:8], AF.Square, scale=1.0 / DI)
        var8 = spool.tile([128, 8], F32)
        nc.vector.scalar_tensor_tensor(var8[:], st128[:, 8:16], 1.0 / DI,
                                       msq[:], OP.mult, OP.subtract)
        eps_sb = spool.tile([128, 1], F32)
        nc.vector.memset(eps_sb[:], EPS)
        sd8 = spool.tile([128, 8], F32)
        nc.scalar.activation(sd8[:], var8[:], AF.Sqrt, bias=eps_sb[:], scale=1.0)
        inv8 = spool.tile([128, 8], F32)
        nc.vector.reciprocal(inv8[:], sd8[:])
        minv = spool.tile([2, L], F32)
        nc.sync.dma_start(minv[0:1, :], mu8[:])
        nc.sync.dma_start(minv[1:2, :], inv8[:])

        # broadcast mu/inv across partitions via PE
        t1 = spool.tile([DH, L], BF)
        for h in range(2):
            mmf = ppool.tile([128, 512], F32, tag=("ping0","ping1")[h],
                             name=f"mu{h}")
            mm = mmf[0:DH, :]
            nc.tensor.matmul(mm, sel2_sb[:, 0:DH],
                             minv[:, h * 512:(h + 1) * 512],
                             start=True, stop=True)
            nc.vector.tensor_tensor(t1[:, h * 512:(h + 1) * 512],
                                    y_full[:, h * 512:(h + 1) * 512],
                                    mm, OP.subtract)
        t2 = spool.tile([DH, L], BF)
        for h in range(2):
            iif = ppool.tile([128, 512], F32, tag=("mm0","mm1")[h],
                             name=f"iv{h}")
            ii = iif[0:DH, :]
            nc.tensor.matmul(ii, sel2_sb[:, DH:2 * DH],
                             minv[:, h * 512:(h + 1) * 512],
                             start=True, stop=True)
            nc.vector.scalar_tensor_tensor(t2[:, h * 512:(h + 1) * 512],
                                           t1[:, h * 512:(h + 1) * 512],
                                           gam_sb[:], ii, OP.mult, OP.mult)
        t3 = spool.tile([DH, L], BF)
        nc.vector.scalar_tensor_tensor(t3[:], t2[:], bet_sb[:], sg[:],
                                       OP.add, OP.mult)

        o_sb = espp.tile([DM, L], F32, tag="esp")
        for h in range(2):
            oof = ppool.tile([128, 512], F32, tag=("ping0", "ping1")[h],
                             name=f"o{h}")
            oo = oof[0:DM, :]
            nc.tensor.matmul(oo, wout_sb[:],
                             t3[:, h * 512:(h + 1) * 512],
                             start=True, stop=True)
            nc.scalar.activation(o_sb[:, h * 512:(h + 1) * 512], oo, AF.Copy)
            nc.sync.dma_start(out_part[:, h * 512:(h + 1) * 512],
                              o_sb[:, h * 512:(h + 1) * 512])

    nc.finalize()
    return nc


def _prep_inputs(inputs):
    """Build the 8 per-core input maps. Core c: b = c//2, dh = c%2."""
    x = np.asarray(inputs["x"], np.float32)
    in_proj_w = np.asarray(inputs["in_proj_w"], np.float32)
    conv_w = np.asarray(inputs["conv_w"], np.float32)
    conv_b = np.asarray(inputs["conv_b"], np.float32)
    xpw = np.asarray(inputs["x_proj_weight"], np.float32)
    dtw = np.asarray(inputs["dt_projs_weight"], np.float32)
    dtb = np.asarray(inputs["dt_projs_bias"], np.float32)
    A_logs = np.asarray(inputs["A_logs"], np.float32)
    Ds = np.asarray(inputs["Ds"], np.float32)
    gam = np.asarray(inputs["ln_gamma"], np.float32)
    bet = np.asarray(inputs["ln_beta"], np.float32)
    wout = np.asarray(inputs["out_proj_w"], np.float32)

    xTf = x.reshape(B, L, DM).transpose(0, 2, 1).copy()      # (B, 96, 1024)
    w_in_T = in_proj_w.T.copy()                               # (96, 384)
    convw9 = conv_w.reshape(DI, 9)                            # (192, 9)
    A = -np.exp(A_logs).reshape(K, DI, NS)                    # (K, 192, 16)
    Dsum_full = Ds.reshape(K, DI).sum(0)                      # (192,)

    SP = store_perm()                                         # [96] perm

    # bcm: delta broadcast lhsT. out row q=(j,n) of tile t reads delta
    # storage-row p with SP[p] == 8t + j.
    inv = np.zeros(DH, np.int64)
    inv[SP] = np.arange(DH)                                   # inv[c] = p
    bcm = np.zeros((DH, NT * 128), np.float32)
    red = np.zeros((128, NT * DH), np.float32)
    for t in range(NT):
        for q in range(128):
            j, n = q // 16, q % 16
            p = inv[8 * t + j]
            bcm[p, t * 128 + q] = 1.0
            red[q, t * DH + p] = 1.0
    ones96 = np.ones((DH, 2), np.float32)
    sel2 = np.zeros((2, 2 * DH), np.float32)
    sel2[0, 0:DH] = 1.0
    sel2[1, DH:2 * DH] = 1.0

    in_maps = []
    for c in range(8):
        b, dh = c // 2, c % 2
        ds_i = np.arange(dh * DH, (dh + 1) * DH)
        other_i = np.arange((1 - dh) * DH, (2 - dh) * DH)
        perm_i = ds_i[SP]            # global channel index at storage p
        # half-0 (this core's half) channel-permuted; half-1 plain.
        w_xi = np.concatenate([w_in_T[:, perm_i], w_in_T[:, other_i]], axis=1)
        convd_r = np.zeros((DH, 2 * 9 * DH), np.float32)
        for cblk, sl in enumerate((perm_i, other_i)):
            taps = convw9[sl]                                 # (96, 9)
            for tap in range(9):
                w0 = (cblk * 9 + tap) * DH
                convd_r[:, w0:w0 + DH][np.arange(DH), np.arange(DH)] = \
                    taps[:, tap]
        convb_r = np.stack([conv_b[perm_i], conv_b[other_i]], axis=1)
        xpw_r = np.zeros((DH, K * 2 * 64), np.float32)
        for k in range(K):
            wk = xpw[k].T  # (192, 38)
            for cblk, sl in enumerate((perm_i, other_i)):
                w0 = (k * 2 + cblk) * 64
                xpw_r[:, w0:w0 + RD] = wk[sl][:, 0:RD]
                xpw_r[:, w0 + 32:w0 + 64] = wk[sl][:, RD:RD + 2 * NS]
        dtw_r = np.zeros((RD, K * DH), np.float32)
        for k in range(K):
            dtw_r[:, k * DH:(k + 1) * DH] = dtw[k][perm_i, :].T
        dtb_r = dtb.reshape(K, DI)[:, perm_i].T.copy()        # (96, K)
        app = np.zeros((128, K * NT), np.float32)
        for k in range(K):
            for t in range(NT):
                for q in range(128):
                    app[q, k * NT + t] = A[k, dh * DH + 8 * t + q // 16,
                                           q % 16]
        in_maps.append({
            "xT": xTf[b].astype(BF_NP),
            "w_xi": w_xi.astype(BF_NP),
            "w_z": w_in_T[:, DI + dh * DH: DI + (dh + 1) * DH][:, SP]
                   .astype(BF_NP),
            "convd": convd_r.astype(BF_NP),
            "convb": convb_r,
            "xpw": xpw_r.astype(BF_NP),
            "dtw": dtw_r.astype(BF_NP),
            "dtb": dtb_r,
            "app": app,
            "bcm": bcm.astype(BF_NP),
            "red": red.astype(BF_NP),
            "dsum": Dsum_full[perm_i][:, None],
            "gam": gam[perm_i][:, None],
            "bet": bet[perm_i][:, None],
            "wout": wout[:, perm_i].T.astype(BF_NP),
            "ones96": ones96.astype(BF_NP),
            "sel2": sel2,
        })
    return in_maps


def kernel(**inputs):
    global _NC
    if _NC is None:
        _NC = build()
    in_maps = _prep_inputs(inputs)
    res = run_bass_kernel_spmd(_NC, in_maps, list(range(8)))
    out = np.zeros((B, L, DM), np.float32)
    for b in range(B):
        part = res.results[2 * b]["out_part"] + res.results[2 * b + 1]["out_part"]
        out[b] = part.T
    return out.reshape(B, HH, WW, DM)
